# revision 10
# baseline (speedup 1.0000x reference)
"""Trainium2 Bass kernel for BlockDiagMNIST MLP.

Reference computation (all fp32):
    h  = relu(x @ W1.T + b1)          x:[B,784], W1:[4096,784]    -> [B,4096]
    yb = blockdiag(h, Wb)             Wb:[128,32,32] (h2[b, 32n+o] = sum_k h[b,32n+k] Wb[n,o,k])
    h2 = relu(yb + bb)
    out = h2 @ W3.T + b3              W3:[10,4096]                -> [B,10]

Strategy: pure data-parallel over batch (B=32768 -> 4096 rows/core on 8 cores),
weights replicated.  All matmuls in bf16 (fp32 PSUM accumulation, fp32 biases).
On-chip layout is transposed ("hidden on partitions"): we compute
hT = W1 @ x.T per 512-column batch window, so layer-2's block-diagonal weight
packs 4 diagonal 32x32 blocks into dense 128x128 matmuls and layer-3 streams
full 512-column matmuls with K=128 chunks.

Host-side prep (free — not on the device timeline): transpose + bf16-cast of
x and weights, packing Wb into 128x128 block-diagonal lhsT tiles.
"""

import numpy as np
import ml_dtypes

B = 32768
IN_DIM = 784
HIDDEN = 4096
BLOCK = 32
NUM_BLOCKS = 128
OUT_DIM = 10
NCORES = 8
BC = B // NCORES          # batch rows per core (4096)
WN = 512                  # batch-window columns (one matmul free-dim)
K1 = 7                    # ceil(784/128) K-chunks for layer 1 (zero-padded to 896)
NM = HIDDEN // 128        # 32 hidden tiles (also layer-2 groups, layer-3 K-chunks)

BF16 = ml_dtypes.bfloat16

_PROGRAM_CACHE = {}


def _build_program(bc=BC):
    """Build (and bacc-compile) the per-core Bass program. bc = batch cols/core."""
    import concourse.mybir as mybir
    import concourse.tile as tile
    from concourse import bacc

    nw = bc // WN
    f32, bf16 = mybir.dt.float32, mybir.dt.bfloat16

    nc = bacc.Bacc("TRN2", target_bir_lowering=False, debug=False)

    xT = nc.dram_tensor("xT", [K1 * 128, bc], bf16, kind="ExternalInput").ap()
    w1t = nc.dram_tensor("W1T", [K1 * 128, HIDDEN], bf16, kind="ExternalInput").ap()
    wbig = nc.dram_tensor("Wbig", [128, NM * 128], bf16, kind="ExternalInput").ap()
    w3t = nc.dram_tensor("W3T", [128, NM * 128], bf16, kind="ExternalInput").ap()
    # biases packed into one tensor: cols 0..NM-1 = b1, NM..2NM-1 = bb, 2NM = b3
    bcat = nc.dram_tensor("bcat", [128, 2 * NM + 1], f32, kind="ExternalInput").ap()
    outT = nc.dram_tensor("outT", [OUT_DIM, bc], f32, kind="ExternalOutput").ap()

    Relu = mybir.ActivationFunctionType.Relu
    Add = mybir.AluOpType.add
    Max = mybir.AluOpType.max

    MB = 4          # W1T column-block = MB m-tiles (DMA granularity for overlap)
    NJ = NM // MB   # 8 column blocks

    with tile.TileContext(nc) as tc:
        with (
            tc.tile_pool(name="const", bufs=1) as cpool,
            tc.tile_pool(name="xin", bufs=3) as xpool,
            tc.tile_pool(name="hbuf", bufs=6) as hpool,
            tc.tile_pool(name="h2buf", bufs=6) as h2pool,
            tc.tile_pool(name="obuf", bufs=2) as opool,
            tc.tile_pool(name="ps1", bufs=3, space="PSUM") as ps1,
            tc.tile_pool(name="ps2", bufs=2, space="PSUM") as ps2,
            tc.tile_pool(name="ps3", bufs=2, space="PSUM") as ps3,
            tc.tile_pool(name="psw", bufs=1, space="PSUM") as psw,
        ):
            # Small constants first (ACT/DVE need them from iteration 0).
            bc_sb = cpool.tile([128, 2 * NM + 1], f32)
            nc.sync.dma_start(bc_sb[:], bcat)
            b1_sb = bc_sb[:, 0:NM]
            bb_sb = bc_sb[:, NM:2 * NM]
            b3_sb = bc_sb[0:OUT_DIM, 2 * NM:2 * NM + 1]

            # HAM warmup: dummy matmuls on the bias tile while the weight/x
            # DMAs stream in, so the PE clock gate is already at 8/8 (2.4GHz)
            # when the first real matmul issues. Results go to a scratch PSUM
            # bank and are never read.
            pw = psw.tile([65, 65], f32)
            for _ in range(22):
                nc.tensor.matmul(
                    pw[:], bc_sb[:, 0:2 * NM + 1], bc_sb[:, 0:2 * NM + 1],
                    start=True, stop=True,
                )

            xT_r = xT.rearrange("(k p) b -> p k b", p=128)
            w1t_r = w1t.rearrange("(k p) h -> p k h", p=128)

            def load_xt(w):
                """Per-window x tile [128, K1, WN], one 3D DMA (prefetched via bufs)."""
                t = xpool.tile([128, K1, WN], bf16, tag="xt", name=f"xt_{w}")
                nc.sync.dma_start(t[:], xT_r[:, :, w * WN:(w + 1) * WN])
                return t

            # Window-0 x tile before the bulk weight load so PE starts early.
            xts = {0: load_xt(0)}

            # W1T as NJ column blocks [128, K1, MB*128], one 3D DMA each; the
            # first block (m-tiles 0..MB-1) lands ahead of everything else.
            w1t_t = [None] * NJ
            for j in range(NJ):
                t = cpool.tile([128, K1, MB * 128], bf16, name=f"w1t_{j}")
                nc.sync.dma_start(
                    t[:], w1t_r[:, :, j * MB * 128:(j + 1) * MB * 128]
                )
                w1t_t[j] = t
                if j == 0:
                    wbig_sb = cpool.tile([128, NM * 128], bf16)
                    nc.sync.dma_start(wbig_sb[:], wbig)
                    w3t_sb = cpool.tile([128, NM * 128], bf16)
                    nc.sync.dma_start(w3t_sb[:], w3t)

            # Software pipeline over global m index: L1(M) | L2(M-DL2) | L3(M-DL3).
            # L2/L3 trail L1 far enough that their semaphore waits (on the
            # ACT/DVE relu evacuations) are already satisfied when the MMs
            # enter the PE queue — an unresolved wait blocks the LDWEIGHTS
            # pull-ahead and costs ~100ns per affected matmul.
            DL2, DL3 = 2, 4
            NTOT = nw * NM
            pos = {}   # window -> psum accumulator for layer 3
            hs = {}    # global M -> h tile (layer-1 output)
            h2s = {}   # global M -> h2 tile (layer-2 output)

            def emit_l1(M):
                w, m = divmod(M, NM)
                if m == 0 and w not in xts:
                    xts[w] = load_xt(w)
                p1 = ps1.tile([128, WN], f32, tag="p1", name=f"p1_{M}")
                for k in range(K1):
                    nc.tensor.matmul(
                        p1[:],
                        w1t_t[m // MB][:, k, (m % MB) * 128:(m % MB + 1) * 128],
                        xts[w][:, k, :],
                        start=(k == 0),
                        stop=(k == K1 - 1),
                    )
                h = hpool.tile([128, WN], bf16, tag="h", name=f"h_{M}")
                nc.scalar.activation(h[:], p1[:], Relu, bias=b1_sb[:, m:m + 1])
                hs[M] = h

            def emit_l2(M):
                w, m = divmod(M, NM)
                p2 = ps2.tile([128, WN], f32, tag="p2", name=f"p2_{M}")
                nc.tensor.matmul(
                    p2[:],
                    wbig_sb[:, m * 128:(m + 1) * 128],
                    hs.pop(M)[:],
                    start=True,
                    stop=True,
                )
                h2 = h2pool.tile([128, WN], bf16, tag="h2", name=f"h2_{M}")
                nc.vector.tensor_scalar(h2[:], p2[:], bb_sb[:, m:m + 1], 0.0, Add, Max)
                h2s[M] = h2

            def emit_l3(M):
                w, m = divmod(M, NM)
                if m == 0:
                    pos[w] = ps3.tile([128, WN], f32, tag="po", name=f"po_{w}")
                nc.tensor.matmul(
                    pos[w][:],
                    w3t_sb[:, m * 128:(m + 1) * 128],
                    h2s.pop(M)[:],
                    start=(m == 0),
                    stop=(m == NM - 1),
                    skip_group_check=True,
                )
                if m == NM - 1:
                    ot = opool.tile([OUT_DIM, WN], f32, tag="ot", name=f"ot_{w}")
                    nc.vector.tensor_scalar_add(ot[:], pos.pop(w)[0:OUT_DIM, :], b3_sb[:])
                    nc.sync.dma_start(outT[:, w * WN:(w + 1) * WN], ot[:])

            for M in range(NTOT + DL3):
                if M < NTOT:
                    emit_l1(M)
                if DL2 <= M < NTOT + DL2:
                    emit_l2(M - DL2)
                if M >= DL3:
                    emit_l3(M - DL3)

    nc.compile()
    return nc


def _get_program(bc=BC):
    if bc not in _PROGRAM_CACHE:
        _PROGRAM_CACHE[bc] = _build_program(bc)
    return _PROGRAM_CACHE[bc]


def _prep_weights(W1, b1, Wb, bb, W3, b3):
    """Host-side packing of replicated weights into device layouts."""
    W1 = np.asarray(W1, dtype=np.float32)
    Wb = np.asarray(Wb, dtype=np.float32)
    W3 = np.asarray(W3, dtype=np.float32)

    # W1T [896, 4096]: row r<784 = W1[:, r]; rows 784.. zero.
    W1T = np.zeros((K1 * 128, HIDDEN), dtype=BF16)
    W1T[:IN_DIM] = W1.T.astype(BF16)

    # Wbig [128, 32*128]: group g holds blockdiag(Wb[4g+j].T), j=0..3.
    Wbig = np.zeros((128, NM * 128), dtype=BF16)
    for g in range(NM):
        for j in range(4):
            blk = Wb[4 * g + j].T.astype(BF16)  # [k, o]
            Wbig[32 * j:32 * j + 32, g * 128 + 32 * j: g * 128 + 32 * j + 32] = blk

    # W3T [128, NM*128]: per m-tile a [128, 128] lhsT whose first OUT_DIM
    # columns are W3[o, 128m+p]; the rest are zero (full-array matmul keeps
    # the PE LDWEIGHTS background-buffer pipeline running; M-cols are free).
    W3T = np.zeros((128, NM * 128), dtype=BF16)
    w3r = W3.reshape(OUT_DIM, NM, 128).transpose(2, 1, 0).astype(BF16)  # [128, NM, 10]
    for mi in range(NM):
        W3T[:, mi * 128:mi * 128 + OUT_DIM] = w3r[:, mi, :]

    bcat = np.zeros((128, 2 * NM + 1), dtype=np.float32)
    bcat[:, 0:NM] = np.asarray(b1, np.float32).reshape(NM, 128).T
    bcat[:, NM:2 * NM] = np.asarray(bb, np.float32).reshape(NM, 128).T
    bcat[0:OUT_DIM, 2 * NM] = np.asarray(b3, np.float32)
    return dict(W1T=W1T, Wbig=Wbig, W3T=W3T, bcat=bcat)


def _prep_x_shard(x, c, ncores=NCORES, bc=BC):
    xs = np.asarray(x[c * bc:(c + 1) * bc], dtype=np.float32)  # [bc, 784]
    xT = np.zeros((K1 * 128, bc), dtype=BF16)
    xT[:IN_DIM] = xs.T.astype(BF16)
    return xT


def run(x, W1, b1, Wb, bb, W3, b3, trace=False, tmpdir=None):
    """Run on 8 cores; returns (out [B,10] fp32, BassKernelResults)."""
    from concourse.bass_utils import run_bass_kernel_spmd

    nc = _get_program()
    wmap = _prep_weights(W1, b1, Wb, bb, W3, b3)
    in_maps = []
    for c in range(NCORES):
        m = dict(wmap)
        m["xT"] = _prep_x_shard(np.asarray(x), c)
        in_maps.append(m)

    res = run_bass_kernel_spmd(
        nc, in_maps, core_ids=list(range(NCORES)), trace=trace, tmpdir=tmpdir
    )
    out = np.concatenate(
        [np.asarray(r["outT"]).T for r in res.results], axis=0
    ).astype(np.float32)
    return out, res


def kernel(x, W1, b1, Wb, bb, W3, b3):
    out, _ = run(x, W1, b1, Wb, bb, W3, b3, trace=False)
    return out


# revision 11
# speedup vs baseline: 1.0577x; 1.0577x over previous
"""Trainium2 Bass kernel for BlockDiagMNIST MLP.

Reference computation (all fp32):
    h  = relu(x @ W1.T + b1)          x:[B,784], W1:[4096,784]    -> [B,4096]
    yb = blockdiag(h, Wb)             Wb:[128,32,32] (h2[b, 32n+o] = sum_k h[b,32n+k] Wb[n,o,k])
    h2 = relu(yb + bb)
    out = h2 @ W3.T + b3              W3:[10,4096]                -> [B,10]

Strategy: pure data-parallel over batch (B=32768 -> 4096 rows/core on 8 cores),
weights replicated.  All matmuls in bf16 (fp32 PSUM accumulation, fp32 biases).
On-chip layout is transposed ("hidden on partitions"): we compute
hT = W1 @ x.T per 512-column batch window, so layer-2's block-diagonal weight
packs 4 diagonal 32x32 blocks into dense 128x128 matmuls and layer-3 streams
full 512-column matmuls with K=128 chunks.

Host-side prep (free — not on the device timeline): transpose + bf16-cast of
x and weights, packing Wb into 128x128 block-diagonal lhsT tiles.
"""

import numpy as np
import ml_dtypes

B = 32768
IN_DIM = 784
HIDDEN = 4096
BLOCK = 32
NUM_BLOCKS = 128
OUT_DIM = 10
NCORES = 8
BC = B // NCORES          # batch rows per core (4096)
WN = 512                  # batch-window columns (one matmul free-dim)
K1 = 6                    # full 128-row K-chunks for layer 1 (features 0..767)
KL = 16                   # leftover K rows (features 768..783), row-group packed
NM = HIDDEN // 128        # 32 hidden tiles (also layer-2 groups, layer-3 K-chunks)

BF16 = ml_dtypes.bfloat16

_PROGRAM_CACHE = {}


def _build_program(bc=BC):
    """Build (and bacc-compile) the per-core Bass program. bc = batch cols/core."""
    import concourse.mybir as mybir
    import concourse.tile as tile
    from concourse import bacc

    nw = bc // WN
    f32, bf16 = mybir.dt.float32, mybir.dt.bfloat16

    nc = bacc.Bacc("TRN2", target_bir_lowering=False, debug=False)

    xT = nc.dram_tensor("xT", [K1 * 128, bc], bf16, kind="ExternalInput").ap()
    xL = nc.dram_tensor("xL", [128, bc], bf16, kind="ExternalInput").ap()
    w1t = nc.dram_tensor("W1T", [K1 * 128, HIDDEN], bf16, kind="ExternalInput").ap()
    w1l = nc.dram_tensor("W1L", [128, HIDDEN], bf16, kind="ExternalInput").ap()
    wbig = nc.dram_tensor("Wbig", [128, NM * 128], bf16, kind="ExternalInput").ap()
    w3t = nc.dram_tensor("W3T", [128, NM * 128], bf16, kind="ExternalInput").ap()
    # biases packed into one tensor: cols 0..NM-1 = b1, NM..2NM-1 = bb, 2NM = b3
    bcat = nc.dram_tensor("bcat", [128, 2 * NM + 1], f32, kind="ExternalInput").ap()
    outT = nc.dram_tensor("outT", [OUT_DIM, bc], f32, kind="ExternalOutput").ap()

    Relu = mybir.ActivationFunctionType.Relu
    Add = mybir.AluOpType.add
    Max = mybir.AluOpType.max

    MB = 4          # W1T column-block = MB m-tiles (DMA granularity for overlap)
    NJ = NM // MB   # 8 column blocks

    with tile.TileContext(nc) as tc:
        with (
            tc.tile_pool(name="const", bufs=1) as cpool,
            tc.tile_pool(name="xin", bufs=3) as xpool,
            tc.tile_pool(name="hbuf", bufs=10) as hpool,
            tc.tile_pool(name="h2buf", bufs=10) as h2pool,
            tc.tile_pool(name="obuf", bufs=2) as opool,
            tc.tile_pool(name="ps1", bufs=5, space="PSUM") as ps1,
            tc.tile_pool(name="ps2", bufs=2, space="PSUM") as ps2,
            tc.tile_pool(name="ps3", bufs=1, space="PSUM") as ps3,
        ):
            # Small constants first (ACT/DVE need them from iteration 0).
            bc_sb = cpool.tile([128, 2 * NM + 1], f32)
            nc.sync.dma_start(bc_sb[:], bcat)
            b1_sb = bc_sb[:, 0:NM]
            bb_sb = bc_sb[:, NM:2 * NM]
            b3_sb = bc_sb[0:OUT_DIM, 2 * NM:2 * NM + 1]

            xT_r = xT.rearrange("(k p) b -> p k b", p=128)
            w1t_r = w1t.rearrange("(k p) h -> p k h", p=128)

            def load_xt(w):
                """Per-window x tiles (one 3D DMA + leftover rows), prefetched."""
                t = xpool.tile([128, K1, WN], bf16, tag="xt", name=f"xt_{w}")
                nc.sync.dma_start(t[:], xT_r[:, :, w * WN:(w + 1) * WN])
                tl = xpool.tile([128, WN], bf16, tag="xl", name=f"xl_{w}")
                nc.sync.dma_start(tl[:], xL[:, w * WN:(w + 1) * WN])
                return t, tl

            # Window-0 x tile before the bulk weight load so PE starts early.
            xts = {0: load_xt(0)}

            # W1T as NJ column blocks [128, K1, MB*128], one 3D DMA each; the
            # first block (m-tiles 0..MB-1) lands ahead of everything else.
            w1t_t = [None] * NJ
            for j in range(NJ):
                t = cpool.tile([128, K1, MB * 128], bf16, name=f"w1t_{j}")
                nc.sync.dma_start(
                    t[:], w1t_r[:, :, j * MB * 128:(j + 1) * MB * 128]
                )
                w1t_t[j] = t
                if j == 0:
                    w1l_sb = cpool.tile([128, HIDDEN], bf16)
                    nc.sync.dma_start(w1l_sb[:], w1l)
                    wbig_sb = cpool.tile([128, NM * 128], bf16)
                    nc.sync.dma_start(wbig_sb[:], wbig)
                    w3t_sb = cpool.tile([128, NM * 128], bf16)
                    nc.sync.dma_start(w3t_sb[:], w3t)

            # Software pipeline, emitted in groups of 4 m-tiles:
            #   L1(G) | L2(G-1) | L3(G-2)
            # Trailing L2/L3 keep the ACT/DVE relu evacuations well ahead of
            # the matmuls that consume them. Each group runs 4x6 full-K
            # matmuls, then the four K=16 leftover matmuls packed into
            # distinct 32-row groups of the PE array (tile_position) so they
            # execute concurrently (~1 matmul-time instead of 4).
            NGW = NM // 4
            NGTOT = nw * NGW
            pos = {}   # window -> psum accumulator for layer 3
            hs = {}    # global M -> h tile (layer-1 output)
            h2s = {}   # global M -> h2 tile (layer-2 output)

            def emit_l1_group(G):
                w, g = divmod(G, NGW)
                if g == 0 and w not in xts:
                    xts[w] = load_xt(w)
                xt, xl = xts[w]
                p1s = []
                for j in range(4):
                    m = 4 * g + j
                    p1 = ps1.tile([128, WN], f32, tag="p1", name=f"p1_{G}_{j}")
                    for k in range(K1):
                        nc.tensor.matmul(
                            p1[:],
                            w1t_t[m // MB][:, k, (m % MB) * 128:(m % MB + 1) * 128],
                            xt[:, k, :],
                            start=(k == 0),
                            stop=False,
                        )
                    p1s.append(p1)
                for j in range(4):
                    m = 4 * g + j
                    nc.tensor.matmul(
                        p1s[j][:],
                        w1l_sb[32 * j:32 * j + KL, m * 128:(m + 1) * 128],
                        xl[32 * j:32 * j + KL, :],
                        start=False,
                        stop=True,
                        tile_position=(32 * j, 0),
                        skip_group_check=True,
                    )
                for j in range(4):
                    m = 4 * g + j
                    h = hpool.tile([128, WN], bf16, tag="h", name=f"h_{G}_{j}")
                    nc.scalar.activation(h[:], p1s[j][:], Relu, bias=b1_sb[:, m:m + 1])
                    hs[w * NM + m] = h

            def emit_l2(M):
                w, m = divmod(M, NM)
                p2 = ps2.tile([128, WN], f32, tag="p2", name=f"p2_{M}")
                nc.tensor.matmul(
                    p2[:],
                    wbig_sb[:, m * 128:(m + 1) * 128],
                    hs.pop(M)[:],
                    start=True,
                    stop=True,
                )
                h2 = h2pool.tile([128, WN], bf16, tag="h2", name=f"h2_{M}")
                nc.vector.tensor_scalar(h2[:], p2[:], bb_sb[:, m:m + 1], 0.0, Add, Max)
                h2s[M] = h2

            def emit_l3(M):
                w, m = divmod(M, NM)
                if m == 0:
                    pos[w] = ps3.tile([128, WN], f32, tag="po", name=f"po_{w}")
                nc.tensor.matmul(
                    pos[w][:],
                    w3t_sb[:, m * 128:(m + 1) * 128],
                    h2s.pop(M)[:],
                    start=(m == 0),
                    stop=(m == NM - 1),
                    skip_group_check=True,
                )
                if m == NM - 1:
                    ot = opool.tile([OUT_DIM, WN], f32, tag="ot", name=f"ot_{w}")
                    nc.vector.tensor_scalar_add(ot[:], pos.pop(w)[0:OUT_DIM, :], b3_sb[:])
                    nc.sync.dma_start(outT[:, w * WN:(w + 1) * WN], ot[:])

            for G in range(NGTOT + 2):
                if G < NGTOT:
                    emit_l1_group(G)
                if 1 <= G <= NGTOT:
                    for j in range(4):
                        emit_l2(4 * (G - 1) + j)
                if G >= 2:
                    for j in range(4):
                        emit_l3(4 * (G - 2) + j)

    nc.compile()
    return nc


def _get_program(bc=BC):
    if bc not in _PROGRAM_CACHE:
        _PROGRAM_CACHE[bc] = _build_program(bc)
    return _PROGRAM_CACHE[bc]


def _prep_weights(W1, b1, Wb, bb, W3, b3):
    """Host-side packing of replicated weights into device layouts."""
    W1 = np.asarray(W1, dtype=np.float32)
    Wb = np.asarray(Wb, dtype=np.float32)
    W3 = np.asarray(W3, dtype=np.float32)

    # W1T [768, 4096] = first 768 input features; W1L [128, 4096] holds the
    # 16 leftover feature rows replicated at partition offsets 0/32/64/96 for
    # the row-group-packed leftover matmuls.
    W1T = np.ascontiguousarray(W1.T[:K1 * 128]).astype(BF16)
    W1L = np.zeros((128, HIDDEN), dtype=BF16)
    lo = W1.T[K1 * 128:IN_DIM].astype(BF16)
    for j in range(4):
        W1L[32 * j:32 * j + KL] = lo

    # Wbig [128, 32*128]: group g holds blockdiag(Wb[4g+j].T), j=0..3.
    Wbig = np.zeros((128, NM * 128), dtype=BF16)
    for g in range(NM):
        for j in range(4):
            blk = Wb[4 * g + j].T.astype(BF16)  # [k, o]
            Wbig[32 * j:32 * j + 32, g * 128 + 32 * j: g * 128 + 32 * j + 32] = blk

    # W3T [128, NM*128]: per m-tile a [128, 128] lhsT whose first OUT_DIM
    # columns are W3[o, 128m+p]; the rest are zero (full-array matmul keeps
    # the PE LDWEIGHTS background-buffer pipeline running; M-cols are free).
    W3T = np.zeros((128, NM * 128), dtype=BF16)
    w3r = W3.reshape(OUT_DIM, NM, 128).transpose(2, 1, 0).astype(BF16)  # [128, NM, 10]
    for mi in range(NM):
        W3T[:, mi * 128:mi * 128 + OUT_DIM] = w3r[:, mi, :]

    bcat = np.zeros((128, 2 * NM + 1), dtype=np.float32)
    bcat[:, 0:NM] = np.asarray(b1, np.float32).reshape(NM, 128).T
    bcat[:, NM:2 * NM] = np.asarray(bb, np.float32).reshape(NM, 128).T
    bcat[0:OUT_DIM, 2 * NM] = np.asarray(b3, np.float32)
    return dict(W1T=W1T, W1L=W1L, Wbig=Wbig, W3T=W3T, bcat=bcat)


def _prep_x_shard(x, c, ncores=NCORES, bc=BC):
    xs = np.asarray(x[c * bc:(c + 1) * bc], dtype=np.float32).T.astype(BF16)  # [784, bc]
    xT = np.ascontiguousarray(xs[:K1 * 128])
    xLs = np.zeros((128, bc), dtype=BF16)
    for j in range(4):
        xLs[32 * j:32 * j + KL] = xs[K1 * 128:IN_DIM]
    return xT, xLs


def run(x, W1, b1, Wb, bb, W3, b3, trace=False, tmpdir=None):
    """Run on 8 cores; returns (out [B,10] fp32, BassKernelResults)."""
    from concourse.bass_utils import run_bass_kernel_spmd

    nc = _get_program()
    wmap = _prep_weights(W1, b1, Wb, bb, W3, b3)
    in_maps = []
    for c in range(NCORES):
        m = dict(wmap)
        m["xT"], m["xL"] = _prep_x_shard(np.asarray(x), c)
        in_maps.append(m)

    res = run_bass_kernel_spmd(
        nc, in_maps, core_ids=list(range(NCORES)), trace=trace, tmpdir=tmpdir
    )
    out = np.concatenate(
        [np.asarray(r["outT"]).T for r in res.results], axis=0
    ).astype(np.float32)
    return out, res


def kernel(x, W1, b1, Wb, bb, W3, b3):
    out, _ = run(x, W1, b1, Wb, bb, W3, b3, trace=False)
    return out
